# revision 1
# baseline (speedup 1.0000x reference)
"""DeepFM forward kernel for 8 Trainium2 NeuronCores (Bass/Tile).

Strategy (data-parallel over batch, per the sharding hint):
  - Batch B=16384 split 8 ways -> 2048 rows/core. Embedding table, fc
    table and MLP weights replicated to every core.
  - Embedding rows fetched with the SWDGE dma_gather custom instruction
    (512B row per index); fc values fetched the same way from a 64-wide
    zero-padded view of fc (256B stride requirement), per-field with the
    field's offset folded into the source access pattern so raw int16
    ids can be used.
  - FM row stats computed in f32 from the gathered rows; the gathered
    blocks are transposed on the PE into feature-major bf16 layout.
  - MLP runs feature-major: h_{l+1}T = relu(W_l.T @ h_lT + b) so every
    layer uses the weights' natural [in, out] layout as lhsT and no
    activation transposes are needed. bf16 inputs, f32 PSUM accumulate.
  - The FM quirk term 0.5*sum_B(rowsum^2 - rowssq) is a GLOBAL scalar:
    phase A computes per-core partials, the host sums 8 floats, phase B
    takes the scalar and produces sigmoid(mlp_y + lin + 0.5*g + bias).
  - Output y[b] f32 [16384, 1].
"""

import os
import numpy as np

# ---- problem constants (hardcoded; kernel.py must be self-contained) ----
TOTAL = 38279
CAT_SIZES = [31360, 6807, 18, 94]
EMB = 128
F = 4
B = 16384
N_CORES = 8
P = 128
FCW = 64                      # fc padded row width (256B stride for gather)
OFFSETS_NP = np.array([0, 31360, 38167, 38185], dtype=np.int32)

_build_cache = {}


def _build(b_loc, n_cores, use_gather=True, cast_dma=True, phase="A"):
    """Build + compile the per-core SPMD program (phase "A" or "B")."""
    import concourse.bass as bass
    import concourse.mybir as mybir
    import concourse.tile as tile
    from concourse import bacc

    f32 = mybir.dt.float32
    bf16 = mybir.dt.bfloat16
    i32 = mybir.dt.int32
    AF = mybir.ActivationFunctionType
    ALU = mybir.AluOpType
    AX = mybir.AxisListType

    NCH = b_loc // P                 # 128-row chunks per core
    GSZ = min(4, NCH)                # chunks per gather group
    NG = NCH // GSZ
    NB = min(512, b_loc)             # matmul moving (batch) width
    NJ = b_loc // NB
    CPJ = NB // P                    # chunks per n-chunk
    NIDX = GSZ * F * P               # embedding indices per gather group
    need_fm = phase == "A"
    need_mlp = phase == "B"

    nc = bacc.Bacc(
        "TRN2",
        target_bir_lowering=False,
        debug=False,
        num_devices=n_cores,
    )

    # ---- DRAM I/O ----
    emb_d = nc.dram_tensor("emb_table", [TOTAL, EMB], f32, kind="ExternalInput").ap()
    x_d = nc.dram_tensor("x", [b_loc, F], f32, kind="ExternalInput").ap()
    if need_fm:
        gpart_d = nc.dram_tensor("gpart", [1, 1], f32, kind="ExternalOutput").ap()
    if need_mlp:
        bias_d = nc.dram_tensor("bias", [1, 1], f32, kind="ExternalInput").ap()
        fc_d = nc.dram_tensor("fc", [TOTAL, 1], f32, kind="ExternalInput").ap()
        W1_d = nc.dram_tensor("W1", [512, 2048], f32, kind="ExternalInput").ap()
        W2_d = nc.dram_tensor("W2", [2048, 1024], f32, kind="ExternalInput").ap()
        W3_d = nc.dram_tensor("W3", [1024, 512], f32, kind="ExternalInput").ap()
        W4_d = nc.dram_tensor("W4", [512, 1], f32, kind="ExternalInput").ap()
        b1_d = nc.dram_tensor("b1", [2048], f32, kind="ExternalInput").ap()
        b2_d = nc.dram_tensor("b2", [1024], f32, kind="ExternalInput").ap()
        b3_d = nc.dram_tensor("b3", [512], f32, kind="ExternalInput").ap()
        b4_d = nc.dram_tensor("b4", [1, 1], f32, kind="ExternalInput").ap()
        ident_d = nc.dram_tensor("ident", [P, P], f32, kind="ExternalInput").ap()
        g_ext_d = nc.dram_tensor("g_ext", [1, 1], f32, kind="ExternalInput").ap()
        y_d = nc.dram_tensor("y", [b_loc, 1], f32, kind="ExternalOutput").ap()

    KT1, MT1 = 512 // P, 2048 // P
    KT2, MT2 = 2048 // P, 1024 // P
    KT3, MT3 = 1024 // P, 512 // P
    KT4 = 512 // P

    with tile.TileContext(nc) as tc:
        with (
            tc.tile_pool(name="const", bufs=1) as const,
            tc.tile_pool(name="gat", bufs=2) as gat,
            tc.tile_pool(name="work", bufs=2) as work,
            tc.tile_pool(name="acts", bufs=1) as acts,
            tc.tile_pool(name="psmm", bufs=3, space="PSUM") as psum_mm,
            tc.tile_pool(name="pstp", bufs=2, space="PSUM") as psum_tp,
            tc.tile_pool(name="psl4", bufs=1, space="PSUM") as psum_l4,
            tc.tile_pool(name="psmisc", bufs=1, space="PSUM") as psum_misc,
        ):
            # ---- raw ids (int32) for the per-(chunk,field) gathers ----
            xw = const.tile([P, NCH, F], f32, tag="xw")
            nc.sync.dma_start(xw[:], x_d.rearrange("(c p) f -> p c f", p=P))
            xi = const.tile([P, NCH, F], i32, tag="xi")
            nc.vector.tensor_copy(xi[:], xw[:])

            if need_mlp:
                ident = const.tile([P, P], f32, tag="ident")
                nc.sync.dma_start(ident[:], ident_d)
                bias_sb = const.tile([1, 1], f32, tag="bias_sb")
                nc.sync.dma_start(bias_sb[:], bias_d)
                b4_sb = const.tile([1, 1], f32, tag="b4_sb")
                nc.sync.dma_start(b4_sb[:], b4_d)
                ones_row = const.tile([1, P], f32, tag="ones_row")
                nc.vector.memset(ones_row[:], 1.0)

                # ---- weights (DMA-cast f32 -> bf16 via SWDGE) ----
                def load_w(dst, src):
                    if cast_dma:
                        nc.gpsimd.dma_start(dst, src)
                    else:
                        stg = work.tile(list(dst.shape), f32, tag="wstage",
                                        name="wstage")
                        nc.sync.dma_start(stg[:], src)
                        nc.vector.tensor_copy(dst, stg[:])

                W1b = [const.tile([P, 2048], bf16, tag=f"w1_{k}", name=f"w1_{k}")
                       for k in range(KT1)]
                for k in range(KT1):
                    load_w(W1b[k][:], W1_d[k * P:(k + 1) * P, :])
                W2b = [const.tile([P, 1024], bf16, tag=f"w2_{k}", name=f"w2_{k}")
                       for k in range(KT2)]
                for k in range(KT2):
                    load_w(W2b[k][:], W2_d[k * P:(k + 1) * P, :])
                W3b = [const.tile([P, 512], bf16, tag=f"w3_{k}", name=f"w3_{k}")
                       for k in range(KT3)]
                for k in range(KT3):
                    load_w(W3b[k][:], W3_d[k * P:(k + 1) * P, :])
                W4b = const.tile([P, KT4], bf16, tag="w4")
                load_w(W4b[:], W4_d.rearrange("(k p) o -> p (k o)", p=P))

                # ---- biases, partition-major per m-tile ----
                b1_sb = const.tile([P, MT1], f32, tag="b1_sb")
                nc.sync.dma_start(b1_sb[:], b1_d.rearrange("(m p) -> p m", p=P))
                b2_sb = const.tile([P, MT2], f32, tag="b2_sb")
                nc.sync.dma_start(b2_sb[:], b2_d.rearrange("(m p) -> p m", p=P))
                b3_sb = const.tile([P, MT3], f32, tag="b3_sb")
                nc.sync.dma_start(b3_sb[:], b3_d.rearrange("(m p) -> p m", p=P))

                # ---- fc gathers: production-shaped [P,1]-index indirect DMA,
                # one per (chunk, field); the field offset goes in
                # element_offset so raw ids are used directly ----
                fcv = const.tile([P, NCH, F], f32, tag="fcv")
                if use_gather:
                    for c in range(NCH):
                        for f in range(F):
                            nc.gpsimd.indirect_dma_start(
                                out=fcv[:, c, f:f + 1],
                                out_offset=None,
                                in_=fc_d,
                                in_offset=bass.IndirectOffsetOnAxis(
                                    ap=xi[:, c, f:f + 1], axis=0
                                ),
                                element_offset=int(OFFSETS_NP[f]),
                            )
                else:
                    nc.vector.memset(fcv[:], 0.25)
                lin = const.tile([P, NCH], f32, tag="lin")
                nc.vector.reduce_sum(out=lin[:], in_=fcv[:], axis=AX.X)

            if need_fm:
                ones_col = const.tile([P, 1], f32, tag="ones_col")
                nc.vector.memset(ones_col[:], 1.0)
                rs4 = const.tile([P, NCH, F], f32, tag="rs4")
                rssq = const.tile([P, NCH], f32, tag="rssq")
            if need_mlp:
                embT = [const.tile([P, b_loc], bf16, tag=f"embT{f}",
                                   name=f"embT{f}") for f in range(F)]

            # ---- embedding gather (+ FM row stats) (+ PE transpose) ----
            for g in range(NG):
                G = gat.tile([P, GSZ * F, EMB], f32, tag="G")
                if use_gather:
                    for cs in range(GSZ):
                        for f in range(F):
                            nc.gpsimd.indirect_dma_start(
                                out=G[:, cs * F + f, :],
                                out_offset=None,
                                in_=emb_d,
                                in_offset=bass.IndirectOffsetOnAxis(
                                    ap=xi[:, g * GSZ + cs, f:f + 1], axis=0
                                ),
                            )
                else:
                    nc.vector.memset(G[:], 0.01)
                if need_fm:
                    nc.vector.reduce_sum(
                        out=rs4[:, g * GSZ:(g + 1) * GSZ, :], in_=G[:], axis=AX.X
                    )
                    # per-chunk sum of squares (square then reduce; the fused
                    # tensor_tensor_reduce op faults the runtime on this stack)
                    for cs in range(GSZ):
                        c = g * GSZ + cs
                        sq = work.tile([P, F * EMB], f32, tag="sqsc")
                        nc.vector.tensor_tensor(
                            out=sq[:],
                            in0=G[:, cs * F:(cs + 1) * F, :],
                            in1=G[:, cs * F:(cs + 1) * F, :],
                            op=ALU.mult,
                        )
                        nc.vector.reduce_sum(
                            out=rssq[:, c:c + 1], in_=sq[:], axis=AX.X
                        )
                if need_mlp:
                    for cs in range(GSZ):
                        c = g * GSZ + cs
                        for f in range(F):
                            tp = psum_tp.tile([P, P], f32, tag="tp")
                            nc.tensor.transpose(tp[:], G[:, cs * F + f, :],
                                                ident[:])
                            nc.vector.tensor_copy(
                                embT[f][:, c * P:(c + 1) * P], tp[:]
                            )

            if need_fm:
                # ---- FM global scalar partial -> gpart ----
                rowsum = const.tile([P, NCH], f32, tag="rowsum")
                nc.vector.reduce_sum(out=rowsum[:], in_=rs4[:], axis=AX.X)
                sosd = const.tile([P, NCH], f32, tag="sosd")
                nc.vector.tensor_tensor(
                    out=sosd[:], in0=rowsum[:], in1=rowsum[:], op=ALU.mult
                )
                nc.vector.tensor_tensor(
                    out=sosd[:], in0=sosd[:], in1=rssq[:], op=ALU.subtract
                )
                pg = const.tile([P, 1], f32, tag="pg")
                nc.vector.reduce_sum(out=pg[:], in_=sosd[:], axis=AX.X)
                gps = psum_misc.tile([1, 1], f32, tag="gps")
                nc.tensor.matmul(
                    gps[:], lhsT=pg[:], rhs=ones_col[:], start=True, stop=True
                )
                g_sb = const.tile([1, 1], f32, tag="g_sb")
                nc.vector.tensor_copy(g_sb[:], gps[:])
                nc.sync.dma_start(gpart_d, g_sb[:])

            if need_mlp:
                # S = 0.5*g + bias + b4  (scalar)
                g_all = const.tile([1, 1], f32, tag="g_all")
                nc.sync.dma_start(g_all[:], g_ext_d)
                S1 = const.tile([1, 1], f32, tag="S1")
                nc.scalar.activation(S1[:], g_all[:], AF.Identity,
                                     bias=bias_sb[:], scale=0.5)
                S2 = const.tile([1, 1], f32, tag="S2")
                nc.scalar.activation(S2[:], S1[:], AF.Identity,
                                     bias=b4_sb[:], scale=1.0)
                # broadcast S to all partitions via K=1 ones-matmul
                Sps = psum_misc.tile([P, 1], f32, tag="Sps")
                nc.tensor.matmul(
                    Sps[:], lhsT=ones_row[:], rhs=S2[:], start=True, stop=True
                )
                Sbc = const.tile([P, 1], f32, tag="Sbc")
                nc.vector.tensor_copy(Sbc[:], Sps[:])
                linS = const.tile([P, NCH], f32, tag="linS")
                nc.vector.tensor_tensor(
                    out=linS[:],
                    in0=lin[:],
                    in1=Sbc[:].to_broadcast([P, NCH]),
                    op=ALU.add,
                )

                # ---- MLP (feature-major) + tail ----
                ysb = const.tile([P, NCH], f32, tag="ysb")
                layers = [
                    (KT1, MT1, W1b, b1_sb, "h1"),
                    (KT2, MT2, W2b, b2_sb, "h2"),
                    (KT3, MT3, W3b, b3_sb, "h3"),
                ]
                for j in range(NJ):
                    jsl = slice(j * NB, (j + 1) * NB)
                    h_prev = [embT[k][:, jsl] for k in range(KT1)]
                    for (KT, MT, Wb, bsb, lname) in layers:
                        h_next = []
                        for m in range(MT):
                            ps = psum_mm.tile([P, NB], f32, tag="mm")
                            for k in range(KT):
                                nc.tensor.matmul(
                                    ps[:],
                                    lhsT=Wb[k][:, m * P:(m + 1) * P],
                                    rhs=h_prev[k],
                                    start=(k == 0),
                                    stop=(k == KT - 1),
                                )
                            t = acts.tile([P, NB], bf16, tag=f"{lname}_{m}",
                                          name=f"{lname}_{m}_{j}")
                            nc.scalar.activation(
                                t[:], ps[:], AF.Relu, bias=bsb[:, m:m + 1]
                            )
                            h_next.append(t[:])
                        h_prev = h_next
                    # final layer (N=1) in batch-on-partition layout + sigmoid
                    for cs in range(CPJ):
                        c = j * CPJ + cs
                        ps4 = psum_l4.tile([P, 1], f32, tag="l4")
                        for k in range(KT4):
                            nc.tensor.matmul(
                                ps4[:],
                                lhsT=h_prev[k][:, cs * P:(cs + 1) * P],
                                rhs=W4b[:, k:k + 1],
                                start=(k == 0),
                                stop=(k == KT4 - 1),
                            )
                        nc.scalar.activation(
                            ysb[:, c:c + 1], ps4[:], AF.Sigmoid,
                            bias=linS[:, c:c + 1]
                        )

                nc.sync.dma_start(y_d.rearrange("(c p) o -> p (c o)", p=P),
                                  ysb[:])

    nc.compile()
    return nc


def _get_program(b_loc, n_cores, **kw):
    key = (b_loc, n_cores, tuple(sorted(kw.items())))
    if key not in _build_cache:
        _build_cache[key] = _build(b_loc, n_cores, **kw)
    return _build_cache[key]


def _wrap_idx(lin_idx):
    """lin_idx [n] int -> [128, n//16] int16 dma_gather index tile:
    tile[p, s] = lin_idx[s*16 + p%16] (16-wrap, replicated for 8 Q7 cores)."""
    n = lin_idx.shape[0]
    wrap = lin_idx.astype(np.int16).reshape(n // 16, 16).T  # [16, n//16]
    return np.ascontiguousarray(np.tile(wrap, (8, 1)))


def make_in_maps(inputs, b_loc, n_cores, phase="A", g_ext=None):
    """Host-side sharding/layout: slice x over batch, build int16 gather
    index tiles and the 256B-stride padded fc view; replicate the rest."""
    x_int = np.asarray(inputs["x"], dtype=np.float32).astype(np.int32)
    NCH = b_loc // P
    GSZ = min(4, NCH)
    NG = NCH // GSZ

    shared = {
        "emb_table": np.ascontiguousarray(
            np.asarray(inputs["emb_table"], np.float32)),
    }
    if phase == "B":
        shared.update({
            "fc": np.ascontiguousarray(np.asarray(inputs["fc"], np.float32)),
            "ident": np.eye(P, dtype=np.float32),
            "bias": np.asarray(inputs["bias"], np.float32).reshape(1, 1),
            "W1": np.ascontiguousarray(np.asarray(inputs["W1"], np.float32)),
            "W2": np.ascontiguousarray(np.asarray(inputs["W2"], np.float32)),
            "W3": np.ascontiguousarray(np.asarray(inputs["W3"], np.float32)),
            "W4": np.ascontiguousarray(np.asarray(inputs["W4"], np.float32)),
            "b1": np.ascontiguousarray(np.asarray(inputs["b1"], np.float32)),
            "b2": np.ascontiguousarray(np.asarray(inputs["b2"], np.float32)),
            "b3": np.ascontiguousarray(np.asarray(inputs["b3"], np.float32)),
            "b4": np.asarray(inputs["b4"], np.float32).reshape(1, 1),
            "g_ext": np.asarray(g_ext, np.float32).reshape(1, 1),
        })

    x = np.ascontiguousarray(np.asarray(inputs["x"], dtype=np.float32))
    in_maps = []
    for c in range(n_cores):
        m = dict(shared)
        m["x"] = np.ascontiguousarray(x[c * b_loc:(c + 1) * b_loc])
        in_maps.append(m)
    return in_maps


def kernel(**inputs) -> np.ndarray:
    from concourse.bass_utils import run_bass_kernel_spmd

    n_cores = N_CORES
    b_loc = B // n_cores
    cores = list(range(n_cores))
    trace = bool(int(os.environ.get("KERNEL_TRACE", "0")))

    # Phase A: per-core FM partial scalar
    ncA = _get_program(b_loc, n_cores, phase="A")
    resA = run_bass_kernel_spmd(
        ncA, make_in_maps(inputs, b_loc, n_cores, phase="A"), core_ids=cores,
        trace=trace,
    )
    g = np.float32(0.0)
    for r in resA.results:
        g = np.float32(g + np.float32(r["gpart"][0, 0]))

    # Phase B: MLP + tail with the all-reduced scalar
    ncB = _get_program(b_loc, n_cores, phase="B")
    resB = run_bass_kernel_spmd(
        ncB, make_in_maps(inputs, b_loc, n_cores, phase="B", g_ext=g),
        core_ids=cores, trace=trace,
    )
    kernel._last_results = (resA, resB)
    a_ns = resA.exec_time_ns
    b_ns = resB.exec_time_ns
    kernel._last_exec_ns = (
        (a_ns or 0) + (b_ns or 0) if (a_ns is not None or b_ns is not None)
        else None
    )
    kernel._last_exec_parts = (a_ns, b_ns)
    out = np.concatenate([r["y"] for r in resB.results], axis=0)
    return out.astype(np.float32)

